# revision 10
# baseline (speedup 1.0000x reference)
"""Trainium2 Bass kernel for nn_HeteDP (GAT-based heterogeneous message passing).

Strategy (8 NeuronCores, SPMD, dst-sharded graph parallelism):
  - Each core owns a contiguous range of destination nodes (N/8 rounded to
    128-blocks). Edges are bucketed by dst on the host (index manipulation
    only); all numerical work runs on device.
  - Phase 0 (dense): every core computes the full per-metapath feature table
    feat = h @ W (plus folded attention projections el = h @ (W@Al),
    er = h @ (W@Ar)) and writes it to its own DRAM as a gather table.
  - Phase 1 (aggregation): per (metapath, dst-block of 128): dma_gather of
    per-edge [feat|el] rows by src and er rows by dst, edge softmax without
    the max-subtraction (values are tiny; exp is safe and the max cancels
    exactly), segment-sum via one-hot matmuls accumulated in PSUM.
  - Phase 2: semantic attention partial sums + AllReduce([1,4]) -> beta.
  - Phase 3: z combine, out = z @ pw + pb, pnorm MLP, invd*beta table write.
  - Phase 4 (atten): dense [N, N] attention matrix, column-sharded by dst.
    Per (src-band 128 x col-tile 256) bucket: recompute per-edge mean
    attention value from gathered el/er/invd rows, build one-hot outer
    products on the fly (iota compare), accumulate tiles with matmuls.
Host only does index prep / weight folding / final concatenation.
"""

import numpy as np

try:
    import concourse.bacc as bacc  # noqa: F401
except Exception:  # pragma: no cover
    import sys

    for p in ("/opt/trn_rl_repo", "/root/.axon_site/_ro/trn_rl_repo"):
        if p not in sys.path:
            sys.path.insert(0, p)

import concourse.bacc as bacc
import concourse.bass as bass
import concourse.mybir as mybir
import concourse.tile as tile
from concourse import bass_utils

F32 = mybir.dt.float32
BF16 = mybir.dt.bfloat16
I16 = mybir.dt.int16
AF = mybir.ActivationFunctionType
OP = mybir.AluOpType

CORES = 8
ECLAMP = 12.0  # exp clamp; real e values are O(1)

# knobs
MM_DT = F32          # dtype for one-hot matmuls (segment sums + atten)
GATHER_BF16 = False  # (future) bf16 gather tables


def _round_up(x, m):
    return (x + m - 1) // m * m


def _pack16(vals, total_slots):
    """Pack an int index list into the [128, total_slots//16] int16 layout
    dma_gather expects (edge i -> partition i%16, col i//16; rows 16..127 zero).
    """
    assert total_slots % 16 == 0
    v = np.zeros(total_slots, np.int16)
    v[: len(vals)] = np.asarray(vals, np.int64).astype(np.int16)
    pat = v.reshape(total_slots // 16, 16).T
    return np.tile(pat, (8, 1)).astype(np.int16)


def _pack128(vals, total_slots, fill=-1.0):
    """Pack per-edge f32 values into [128, total_slots//128] (edge i ->
    partition i%128, col i//128)."""
    assert total_slots % 128 == 0
    v = np.full(total_slots, fill, np.float32)
    v[: len(vals)] = np.asarray(vals, np.float64).astype(np.float32)
    return v.reshape(total_slots // 128, 128).T.copy()


def _preprocess(inp, n_cores=CORES):
    h = np.asarray(inp["h"], np.float32)
    src = np.asarray(inp["src"], np.int64)
    dst = np.asarray(inp["dst"], np.int64)
    fc_w = np.asarray(inp["fc_w"], np.float32)
    attn_l = np.asarray(inp["attn_l"], np.float32)
    attn_r = np.asarray(inp["attn_r"], np.float32)
    gat_bias = np.asarray(inp["gat_bias"], np.float32)
    sa_w1 = np.asarray(inp["sa_w1"], np.float32)
    sa_b1 = np.asarray(inp["sa_b1"], np.float32)
    sa_w2 = np.asarray(inp["sa_w2"], np.float32)
    pw = np.asarray(inp["pw"], np.float32)
    pb = np.asarray(inp["pb"], np.float32)
    mlp_w1 = np.asarray(inp["mlp_w1"], np.float32)
    mlp_w2 = np.asarray(inp["mlp_w2"], np.float32)
    mlp_b2 = np.asarray(inp["mlp_b2"], np.float32)
    p_values = np.asarray(inp["p_values"], np.float32)

    N, IN_DIM = h.shape
    M, E = src.shape
    FO = fc_w.shape[2]
    H = attn_l.shape[1]
    D = attn_l.shape[2]
    OD = pw.shape[1]
    assert IN_DIM == 128, "kernel assumes IN_DIM=128"
    assert FO == H * D

    RPC = _round_up(-(-N // n_cores), 128)      # dst rows per core (padded)
    NP = RPC * n_cores                          # padded node count
    BPC = RPC // 128                            # dst blocks per core
    SRCB = NP // 128                            # src bands
    CT = 256 if RPC % 256 == 0 else 128         # atten col-tile width
    NCT = RPC // CT

    cfg = dict(N=N, M=M, E=E, FO=FO, H=H, D=D, OD=OD, RPC=RPC, NP=NP,
               BPC=BPC, SRCB=SRCB, CT=CT, NCT=NCT, n_cores=n_cores)

    # ---- weight folding (host, O(weights)) ----
    # Wl[m][i,h] = sum_d fc_w[m][i, h*D+d] * attn_l[m][h,d]
    fw = fc_w.reshape(M, IN_DIM, H, D)
    Wl = np.einsum("mihd,mhd->mih", fw, attn_l).astype(np.float32)  # [M,128,H]
    Wr = np.einsum("mihd,mhd->mih", fw, attn_r).astype(np.float32)
    # rhs_w packed [128, M*(FO+2H)]
    rhs_w = np.concatenate([
        np.concatenate([fc_w[m], Wl[m], Wr[m]], axis=1) for m in range(M)
    ], axis=1).astype(np.float32)
    RW = FO + 2 * H

    u = (sa_w1 @ sa_w2).astype(np.float32)[:, 0]          # [FO]
    c_sa = float(sa_b1 @ sa_w2[:, 0])
    u_b = np.broadcast_to(u, (128, FO)).copy()
    pw_re = pw.reshape(FO // 128, 128, OD).transpose(1, 0, 2).reshape(128, -1).copy()
    pb_b = np.broadcast_to(pb, (128, OD)).copy()
    w1r_b = np.concatenate([
        np.broadcast_to(mlp_w1[k], (128, OD)) for k in range(mlp_w1.shape[0])
    ], axis=1).astype(np.float32)
    w2_sb = mlp_w2.astype(np.float32)                     # [D?,OD] = [64,64]
    b2_b = np.broadcast_to(mlp_b2, (128, OD)).copy()
    bias_b = np.concatenate([
        np.broadcast_to(gat_bias[m], (128, FO)) for m in range(M)
    ], axis=1).astype(np.float32)
    iota128 = np.broadcast_to(np.arange(128, dtype=np.float32), (128, 128)).copy()
    iotaCT = np.broadcast_to(np.arange(CT, dtype=np.float32), (128, CT)).copy()
    ident = np.eye(128, dtype=np.float32)
    ones_col = np.ones((128, 1), np.float32)
    ones_row = np.ones((1, 128), np.float32)

    hTp = np.zeros((128, NP), np.float32)
    hTp[:, :N] = h.T

    # ---- per-core edge bucketing ----
    core_of = dst // RPC  # [M,E]
    agg_buckets = [[[None] * BPC for _ in range(M)] for _ in range(n_cores)]
    max_agg = 1
    for c in range(n_cores):
        for m in range(M):
            sel = np.nonzero(core_of[m] == c)[0]
            d_loc = dst[m][sel] - c * RPC
            b_of = d_loc // 128
            order = np.argsort(b_of, kind="stable")
            sel, b_of = sel[order], b_of[order]
            bounds = np.searchsorted(b_of, np.arange(BPC + 1))
            for b in range(BPC):
                idx = sel[bounds[b]:bounds[b + 1]]
                agg_buckets[c][m][b] = idx
                max_agg = max(max_agg, len(idx))
    E_blk = _round_up(max_agg, 128)
    # sub-gather split: dma_gather breaks on HW above ~1008 idx (64-desc
    # per-engine packet limit); keep each instruction <=896 idx
    subs = []
    off = 0
    while off < E_blk:
        sz = min(896, E_blk - off)
        subs.append((off, sz))
        off += sz
    band_subs = []
    off = 0
    cfg["E_blk"] = E_blk
    cfg["subs"] = subs

    att_buckets = [[[None] * NCT for _ in range(SRCB)] for _ in range(n_cores)]
    max_att = 1
    for c in range(n_cores):
        for m in range(M):
            sel = np.nonzero(core_of[m] == c)[0]
            s_band = src[m][sel] // 128
            ct_i = (dst[m][sel] - c * RPC) // CT
            # group by (band, coltile)
            key = s_band * NCT + ct_i
            order = np.argsort(key, kind="stable")
            sel2, key2 = sel[order], key[order]
            bounds = np.searchsorted(key2, np.arange(SRCB * NCT + 1))
            for sb in range(SRCB):
                for ct in range(NCT):
                    k = sb * NCT + ct
                    idx = sel2[bounds[k]:bounds[k + 1]]
                    cur = att_buckets[c][sb][ct]
                    ent = (np.full(len(idx), m, np.int64), idx)
                    att_buckets[c][sb][ct] = [ent] if cur is None else cur + [ent]
                    # count handled after merge
    for c in range(n_cores):
        for sb in range(SRCB):
            for ct in range(NCT):
                ents = att_buckets[c][sb][ct] or []
                ms = np.concatenate([e[0] for e in ents]) if ents else np.zeros(0, np.int64)
                es = np.concatenate([e[1] for e in ents]) if ents else np.zeros(0, np.int64)
                att_buckets[c][sb][ct] = (ms, es)
                max_att = max(max_att, len(es))
    B_cap = _round_up(max_att, 128)
    BAND = NCT * B_cap           # slots per src band
    while off < BAND:
        sz = min(896, BAND - off)
        band_subs.append((off, sz))
        off += sz
    cfg["B_cap"] = B_cap
    cfg["BAND"] = BAND
    cfg["band_subs"] = band_subs

    # ---- per-core input arrays ----
    in_maps = []
    shared = dict(hTp=hTp, rhs_w=rhs_w, u_b=u_b, pw_re=pw_re, pb_b=pb_b,
                  w1r_b=w1r_b, w2_sb=w2_sb, b2_b=b2_b, bias_b=bias_b,
                  iota128=iota128, iotaCT=iotaCT, ident=ident,
                  ones_col=ones_col, ones_row=ones_row)
    for c in range(n_cores):
        a_src = np.zeros((128, M * BPC * E_blk // 16), np.int16)
        a_er = np.zeros((128, M * BPC * E_blk // 16), np.int16)
        a_dl = np.full((128, M * BPC * E_blk // 128), -1.0, np.float32)
        for m in range(M):
            for b in range(BPC):
                idx = agg_buckets[c][m][b]
                col0 = (m * BPC + b) * E_blk
                # per-sub wrap16 packing
                for (o, sz) in subs:
                    part = idx[o:o + sz]
                    a_src[:, (col0 + o) // 16:(col0 + o + sz) // 16] = \
                        _pack16(src[m][part], sz)
                    a_er[:, (col0 + o) // 16:(col0 + o + sz) // 16] = \
                        _pack16(dst[m][part], sz)
                a_dl[:, col0 // 128:(col0 + E_blk) // 128] = \
                    _pack128(dst[m][idx] - (c * RPC + b * 128), E_blk)
        t_el = np.zeros((128, SRCB * BAND // 16), np.int16)
        t_dv = np.zeros((128, SRCB * BAND // 16), np.int16)
        t_sl = np.full((128, SRCB * BAND // 128), -1.0, np.float32)
        t_dl = np.full((128, SRCB * BAND // 128), -1.0, np.float32)
        for sb in range(SRCB):
            elv = np.zeros(BAND, np.int64)
            dvv = np.zeros(BAND, np.int64)
            slv = np.full(BAND, -1.0, np.float32)
            dlv = np.full(BAND, -1.0, np.float32)
            for ct in range(NCT):
                ms, es = att_buckets[c][sb][ct]
                o = ct * B_cap
                L = len(es)
                elv[o:o + L] = ms * NP + src[ms, es]
                dvv[o:o + L] = ms * RPC + (dst[ms, es] - c * RPC)
                slv[o:o + L] = (src[ms, es] - sb * 128).astype(np.float32)
                dlv[o:o + L] = ((dst[ms, es] - c * RPC) % CT).astype(np.float32)
            for (o, szz) in band_subs:
                c0 = (sb * BAND + o) // 16
                t_el[:, c0:c0 + szz // 16] = _pack16(elv[o:o + szz], szz)
                t_dv[:, c0:c0 + szz // 16] = _pack16(dvv[o:o + szz], szz)
            t_sl[:, sb * BAND // 128:(sb + 1) * BAND // 128] = _pack128(slv, BAND, -1.0)
            t_dl[:, sb * BAND // 128:(sb + 1) * BAND // 128] = _pack128(dlv, BAND, -1.0)

        hT_own = np.zeros((128, RPC), np.float32)
        lo, hi = c * RPC, min((c + 1) * RPC, N)
        if hi > lo:
            hT_own[:, :hi - lo] = h[lo:hi].T
        rowmask = np.zeros((128, BPC), np.float32)
        nreal = max(0, hi - lo)
        fullb = nreal // 128
        rowmask[:, :fullb] = 1.0
        if fullb < BPC and nreal % 128:
            rowmask[:nreal % 128, fullb] = 1.0
        p_sb = np.zeros((128, 2 * BPC), np.float32)
        if hi > lo:
            pv = np.zeros((RPC, 2), np.float32)
            pv[:hi - lo] = p_values[lo:hi]
            p_sb[:] = pv.reshape(BPC, 128, 2).transpose(1, 0, 2).reshape(128, -1)

        im = dict(shared)
        im.update(a_src=a_src, a_er=a_er, a_dl=a_dl, t_el=t_el, t_dv=t_dv,
                  t_sl=t_sl, t_dl=t_dl, hT_own=hT_own, rowmask=rowmask,
                  p_sb=p_sb)
        in_maps.append(im)

    cfg["c_sa"] = c_sa
    cfg["RW"] = RW
    return cfg, in_maps


def _build_program(cfg):
    M, FO, H, D, OD = cfg["M"], cfg["FO"], cfg["H"], cfg["D"], cfg["OD"]
    N, NP, RPC, BPC, SRCB = cfg["N"], cfg["NP"], cfg["RPC"], cfg["BPC"], cfg["SRCB"]
    CT, NCT, E_blk, subs = cfg["CT"], cfg["NCT"], cfg["E_blk"], cfg["subs"]
    B_cap, BAND, RW = cfg["B_cap"], cfg["BAND"], cfg["RW"]
    n_cores = cfg["n_cores"]
    FTW = _round_up(FO + H, 64)        # feat table row width (f32): 576
    ETW = 64                           # el/er table row width

    nc = bacc.Bacc("TRN2", target_bir_lowering=False, debug=False)

    def din(name, shape, dt=F32):
        return nc.dram_tensor(name, shape, dt, kind="ExternalInput").ap()

    # inputs
    hTp = din("hTp", [128, NP])
    rhs_w = din("rhs_w", [128, M * RW])
    u_b = din("u_b", [128, FO])
    pw_re = din("pw_re", [128, (FO // 128) * OD])
    pb_b = din("pb_b", [128, OD])
    w1r_b = din("w1r_b", [128, 2 * OD])
    w2_sb = din("w2_sb", [D, OD])
    b2_b = din("b2_b", [128, OD])
    bias_b = din("bias_b", [128, M * FO])
    iota128 = din("iota128", [128, 128])
    iotaCT = din("iotaCT", [128, CT])
    ident = din("ident", [128, 128])
    ones_col = din("ones_col", [128, 1])
    ones_row = din("ones_row", [1, 128])
    a_src = din("a_src", [128, M * BPC * E_blk // 16], I16)
    a_er = din("a_er", [128, M * BPC * E_blk // 16], I16)
    a_dl = din("a_dl", [128, M * BPC * E_blk // 128])
    t_el = din("t_el", [128, SRCB * BAND // 16], I16)
    t_dv = din("t_dv", [128, SRCB * BAND // 16], I16)
    t_sl = din("t_sl", [128, SRCB * BAND // 128])
    t_dl = din("t_dl", [128, SRCB * BAND // 128])
    hT_own = din("hT_own", [128, RPC])
    rowmask = din("rowmask", [128, BPC])
    p_sb_in = din("p_sb", [128, 2 * BPC])

    # outputs
    out_rows = nc.dram_tensor("out_rows", [RPC, OD], F32, kind="ExternalOutput").ap()
    atten_loc = nc.dram_tensor("atten_loc", [NP, RPC], F32, kind="ExternalOutput").ap()
    pn_out = nc.dram_tensor("pn_out", [128, BPC], F32, kind="ExternalOutput").ap()

    # DRAM scratch
    feat_t = [nc.dram_tensor(f"feat_t{m}", [NP, FTW], F32).ap() for m in range(M)]
    er_t = [nc.dram_tensor(f"er_t{m}", [NP, ETW], F32).ap() for m in range(M)]
    el_t = nc.dram_tensor("el_t", [M * NP, ETW], F32).ap()
    dst_t = nc.dram_tensor("dst_t", [M * RPC, ETW], F32).ap()
    cc_in = nc.dram_tensor("cc_in", [1, 4], F32).ap()
    cc_out = nc.dram_tensor("cc_out", [1, 4], F32, addr_space="Shared").ap()

    with tile.TileContext(nc) as tc:
        with (
            tc.tile_pool(name="const", bufs=1) as constp,
            tc.tile_pool(name="persist", bufs=1) as persist,
            tc.tile_pool(name="psA", bufs=2, space="PSUM") as psA,
            tc.tile_pool(name="psB", bufs=2, space="PSUM") as psB,
        ):
            # ---- load constants ----
            def load_const(ap, shape, dt=F32):
                t = constp.tile(shape, dt, tag=f"c_{ap.tensor.name}")
                nc.sync.dma_start(t[:], ap)
                return t

            rw_sb = load_const(rhs_w, [128, M * RW])
            ub_sb = load_const(u_b, [128, FO])
            pwre_sb = load_const(pw_re, [128, (FO // 128) * OD])
            pbb_sb = load_const(pb_b, [128, OD])
            w1r_sb = load_const(w1r_b, [128, 2 * OD])
            w2s_sb = load_const(w2_sb, [D, OD])
            b2b_sb = load_const(b2_b, [128, OD])
            biasb_sb = load_const(bias_b, [128, M * FO])
            io128_sb = load_const(iota128, [128, 128])
            ioCT_sb = load_const(iotaCT, [128, CT])
            id_sb = load_const(ident, [128, 128])
            ones_c = load_const(ones_col, [128, 1])
            ones_r = load_const(ones_row, [1, 128])
            mask_sb = load_const(rowmask, [128, BPC])
            psb_sb = load_const(p_sb_in, [128, 2 * BPC])

            z_sb = persist.tile([128, M * BPC * FO], F32, tag="zstore")
            invd_sb = persist.tile([128, M * BPC * H], F32, tag="invd")
            qsum_sb = persist.tile([128, 4], F32, tag="qsum")
            nc.vector.memset(qsum_sb[:], 0.0)

            # ================= PHASE 0: dense tables =================
            with (
                tc.tile_pool(name="p0", bufs=3) as p0,
                tc.tile_pool(name="p0lhs", bufs=3) as p0l,
            ):
                for m in range(M):
                    for sbk in range(SRCB):
                        lhsT = p0l.tile([128, 128], F32, tag="lhs")
                        nc.sync.dma_start(lhsT[:], hTp[:, sbk * 128:(sbk + 1) * 128])
                        ps_f = psA.tile([128, FO], F32, tag="A")
                        ps_le = psB.tile([128, 2 * H], F32, tag="B")
                        nc.tensor.matmul(ps_f[:], lhsT[:], rw_sb[:, m * RW:m * RW + FO],
                                         start=True, stop=True)
                        nc.tensor.matmul(ps_le[:], lhsT[:],
                                         rw_sb[:, m * RW + FO:(m + 1) * RW],
                                         start=True, stop=True)
                        ft = p0.tile([128, FTW], F32, tag="ft")
                        nc.gpsimd.memset(ft[:, FO + H:FTW], 0.0)
                        nc.vector.tensor_copy(ft[:, :FO], ps_f[:])
                        nc.vector.tensor_copy(ft[:, FO:FO + H], ps_le[:, 0:H])
                        nc.sync.dma_start(
                            feat_t[m][sbk * 128:(sbk + 1) * 128, :], ft[:])
                        elrow = p0.tile([128, ETW], F32, tag="elrow")
                        nc.gpsimd.memset(elrow[:, H:ETW], 0.0)
                        nc.vector.tensor_copy(elrow[:, 0:H], ps_le[:, 0:H])
                        nc.sync.dma_start(
                            el_t[m * NP + sbk * 128: m * NP + (sbk + 1) * 128, :],
                            elrow[:])
                        errow = p0.tile([128, ETW], F32, tag="errow")
                        nc.gpsimd.memset(errow[:, H:ETW], 0.0)
                        nc.vector.tensor_copy(errow[:, 0:H], ps_le[:, H:2 * H])
                        nc.sync.dma_start(
                            er_t[m][sbk * 128:(sbk + 1) * 128, :], errow[:])
                # phase 0b: own-row er into dst_t
                for m in range(M):
                    for b in range(BPC):
                        lhsT = p0l.tile([128, 128], F32, tag="lhs")
                        nc.sync.dma_start(lhsT[:], hT_own[:, b * 128:(b + 1) * 128])
                        ps_le = psB.tile([128, 2 * H], F32, tag="B")
                        nc.tensor.matmul(ps_le[:], lhsT[:],
                                         rw_sb[:, m * RW + FO:(m + 1) * RW],
                                         start=True, stop=True)
                        ero = p0.tile([128, ETW], F32, tag="ero")
                        nc.gpsimd.memset(ero[:, H:ETW], 0.0)
                        nc.vector.tensor_copy(ero[:, 0:H], ps_le[:, H:2 * H])
                        nc.sync.dma_start(
                            dst_t[m * RPC + b * 128: m * RPC + (b + 1) * 128, :],
                            ero[:])

            # ================= PHASE 1: aggregation =================
            with (
                tc.tile_pool(name="aggidx", bufs=1) as aggidx,
                tc.tile_pool(name="p1", bufs=2) as p1,
                tc.tile_pool(name="p1s", bufs=2) as p1s,
            ):
                asrc_sb = aggidx.tile([128, M * BPC * E_blk // 16], I16)
                aer_sb = aggidx.tile([128, M * BPC * E_blk // 16], I16)
                adl_sb = aggidx.tile([128, M * BPC * E_blk // 128], F32)
                nc.sync.dma_start(asrc_sb[:], a_src)
                nc.sync.dma_start(aer_sb[:], a_er)
                nc.sync.dma_start(adl_sb[:], a_dl)

                for m in range(M):
                    for b in range(BPC):
                        ps_f = psA.tile([128, FO], F32, tag="A")
                        ps_e = psB.tile([128, H], F32, tag="B")
                        col0 = (m * BPC + b) * E_blk
                        nch_total = E_blk // 128
                        kglob = 0
                        for (o, sz) in subs:
                            nsl = sz // 128
                            gf = p1.tile([128, 1024 // 128, FTW], F32, tag="gf")
                            ge = p1s.tile([128, 1024 // 128, ETW], F32, tag="ge")
                            ee = p1s.tile([128, 1024 // 128, H], F32, tag="ee")
                            nc.gpsimd.dma_gather(
                                gf[:, :nsl, :], feat_t[m],
                                asrc_sb[:, (col0 + o) // 16:(col0 + o + sz) // 16],
                                num_idxs=sz, num_idxs_reg=sz, elem_size=FTW)
                            nc.gpsimd.dma_gather(
                                ge[:, :nsl, :], er_t[m],
                                aer_sb[:, (col0 + o) // 16:(col0 + o + sz) // 16],
                                num_idxs=sz, num_idxs_reg=sz, elem_size=ETW)
                            # e = lrelu(el_src + er_dst, 0.2); ee = exp(min(e,C))
                            et = ee[:, :nsl, :]
                            nc.vector.tensor_tensor(
                                et, gf[:, :nsl, FO:FO + H], ge[:, :nsl, 0:H],
                                op=OP.add)
                            nc.vector.scalar_tensor_tensor(
                                et, et, 0.2, et, op0=OP.mult, op1=OP.max)
                            nc.vector.tensor_scalar_min(et, et, ECLAMP)
                            nc.scalar.activation(et, et, AF.Exp)
                            # wfeat = feat * ee (broadcast over D), in place
                            wf = gf[:, :nsl, :FO].rearrange(
                                "p s (h d) -> p s h d", h=H)
                            eeb = et.unsqueeze(-1).broadcast_to(
                                [128, nsl, H, D])
                            nc.vector.tensor_tensor(wf, wf, eeb, op=OP.mult)
                            for kc in range(nsl):
                                k = kglob + kc
                                oh = p1s.tile([128, 128], MM_DT, tag="oh")
                                nc.vector.tensor_scalar(
                                    oh[:], io128_sb[:],
                                    adl_sb[:, col0 // 128 + k:col0 // 128 + k + 1],
                                    None, op0=OP.is_equal)
                                nc.tensor.matmul(
                                    ps_f[:], oh[:], gf[:, kc, :FO],
                                    start=(k == 0), stop=(k == nch_total - 1))
                                nc.tensor.matmul(
                                    ps_e[:], oh[:], ee[:, kc, :],
                                    start=(k == 0), stop=(k == nch_total - 1))
                            kglob += nsl
                        # denom -> invd
                        sl = (m * BPC + b) * H
                        dn = p1s.tile([128, H], F32, tag="dn")
                        nc.vector.tensor_scalar_max(dn[:], ps_e[:], 1e-30)
                        nc.vector.reciprocal(invd_sb[:, sl:sl + H], dn[:])
                        # rst = elu(psum_f * invd + bias) -> z
                        zv = z_sb[:, (m * BPC + b) * FO:(m * BPC + b + 1) * FO]
                        for hh in range(H):
                            nc.vector.tensor_scalar(
                                zv[:, hh * D:(hh + 1) * D],
                                ps_f[:, hh * D:(hh + 1) * D],
                                invd_sb[:, sl + hh:sl + hh + 1], None, op0=OP.mult)
                        nc.vector.tensor_tensor(
                            zv, zv, biasb_sb[:, m * FO:(m + 1) * FO], op=OP.add)
                        # elu: z = max(x,0) + exp(min(x,0)) - 1
                        t0 = p1s.tile([128, FO], F32, tag="elu")
                        nc.vector.tensor_scalar_min(t0[:], zv, 0.0)
                        nc.scalar.activation(t0[:], t0[:], AF.Exp)
                        nc.vector.scalar_tensor_tensor(
                            zv, zv, 0.0, t0[:], op0=OP.max, op1=OP.add)
                        nc.vector.tensor_scalar_add(zv, zv, -1.0)
                        # semantic partial: q = sum(z*u) ; ql = lrelu(q+c_sa,.01)
                        qt = p1s.tile([128, FO], F32, tag="qt")
                        nc.vector.tensor_tensor(qt[:], zv, ub_sb[:], op=OP.mult)
                        qc = p1s.tile([128, 1], F32, tag="qc")
                        nc.vector.tensor_reduce(qc[:], qt[:],
                                                axis=mybir.AxisListType.X, op=OP.add)
                        nc.vector.tensor_scalar_add(qc[:], qc[:], cfg["c_sa"])
                        nc.vector.scalar_tensor_tensor(
                            qc[:], qc[:], 0.01, qc[:], op0=OP.mult, op1=OP.max)
                        nc.vector.tensor_scalar(
                            qc[:], qc[:], mask_sb[:, b:b + 1], None, op0=OP.mult)
                        nc.vector.tensor_tensor(
                            qsum_sb[:, m:m + 1], qsum_sb[:, m:m + 1], qc[:],
                            op=OP.add)

            # ============ PHASE 2: beta via AllReduce ============
            with tc.tile_pool(name="p2", bufs=1) as p2:
                ps_q = psB.tile([1, 4], F32, tag="B")
                for m in range(M):
                    nc.tensor.matmul(ps_q[:, m:m + 1], ones_c[:],
                                     qsum_sb[:, m:m + 1],
                                     start=True, stop=True)
                qrow = p2.tile([1, 4], F32)
                nc.vector.memset(qrow[:], 0.0)
                nc.vector.tensor_copy(qrow[:, :M], ps_q[:, :M])
                nc.sync.dma_start(cc_in, qrow[:])
                nc.gpsimd.collective_compute(
                    "AllReduce", OP.add,
                    replica_groups=[list(range(n_cores))],
                    ins=[cc_in], outs=[cc_out])
                wrow = p2.tile([1, 4], F32)
                nc.sync.dma_start(wrow[:], cc_out)
                # beta = softmax(w / N) over first M cols
                wm = p2.tile([1, 1], F32)
                nc.vector.tensor_scalar_mul(wrow[:, :M], wrow[:, :M], 1.0 / N)
                nc.vector.tensor_reduce(wm[:], wrow[:, :M],
                                        axis=mybir.AxisListType.X, op=OP.max)
                nc.vector.tensor_scalar(wrow[:, :M], wrow[:, :M], wm[:], None,
                                        op0=OP.subtract)
                nc.scalar.activation(wrow[:, :M], wrow[:, :M], AF.Exp)
                ws = p2.tile([1, 1], F32)
                nc.vector.tensor_reduce(ws[:], wrow[:, :M],
                                        axis=mybir.AxisListType.X, op=OP.add)
                nc.vector.reciprocal(ws[:], ws[:])
                nc.vector.tensor_scalar(wrow[:, :M], wrow[:, :M], ws[:], None,
                                        op0=OP.mult)
                # broadcast beta to all partitions
                ps_b = psB.tile([128, 4], F32, tag="B")
                nc.tensor.matmul(ps_b[:], ones_r[:], wrow[:], start=True, stop=True)
                beta_b = persist.tile([128, 4], F32, tag="beta")
                nc.vector.tensor_copy(beta_b[:], ps_b[:])

            # ============ PHASE 3: z-combine, out, invd', pnorm ============
            with tc.tile_pool(name="p3", bufs=2) as p3:
                # invd' = invd * beta_m / H  -> dst_t cols [H:2H]
                for m in range(M):
                    for b in range(BPC):
                        sl = (m * BPC + b) * H
                        iv = p3.tile([128, H], F32, tag="iv")
                        nc.vector.tensor_scalar(
                            iv[:], invd_sb[:, sl:sl + H],
                            beta_b[:, m:m + 1], 1.0 / H, op0=OP.mult, op1=OP.mult)
                        nc.sync.dma_start(
                            dst_t[m * RPC + b * 128:m * RPC + (b + 1) * 128,
                                  H:2 * H], iv[:])
                for b in range(BPC):
                    zc = p3.tile([128, FO], F32, tag="zc")
                    t1 = p3.tile([128, FO], F32, tag="t1")
                    nc.vector.tensor_scalar(
                        zc[:], z_sb[:, b * FO:(b + 1) * FO],
                        beta_b[:, 0:1], None, op0=OP.mult)
                    for m in range(1, M):
                        nc.vector.tensor_scalar(
                            t1[:], z_sb[:, (m * BPC + b) * FO:(m * BPC + b + 1) * FO],
                            beta_b[:, m:m + 1], None, op0=OP.mult)
                        nc.vector.tensor_tensor(zc[:], zc[:], t1[:], op=OP.add)
                    # out = zT @ pw + pb
                    ps_o = psA.tile([128, OD], F32, tag="A")
                    for k in range(FO // 128):
                        ps_t = psB.tile([128, 128], F32, tag="B")
                        nc.tensor.transpose(ps_t[:], zc[:, k * 128:(k + 1) * 128],
                                            id_sb[:])
                        zT = p3.tile([128, 128], F32, tag="zT")
                        nc.vector.tensor_copy(zT[:], ps_t[:])
                        nc.tensor.matmul(ps_o[:], zT[:],
                                         pwre_sb[:, k * OD:(k + 1) * OD],
                                         start=(k == 0), stop=(k == FO // 128 - 1))
                    ot = p3.tile([128, OD], F32, tag="ot")
                    nc.vector.tensor_tensor(ot[:], ps_o[:], pbb_sb[:], op=OP.add)
                    nc.sync.dma_start(out_rows[b * 128:(b + 1) * 128, :], ot[:])
                    # pnorm
                    y1 = p3.tile([128, OD], F32, tag="y1")
                    y2 = p3.tile([128, OD], F32, tag="y2")
                    nc.vector.tensor_scalar(
                        y1[:], w1r_sb[:, 0:OD], psb_sb[:, 2 * b:2 * b + 1], None,
                        op0=OP.mult)
                    nc.vector.tensor_scalar(
                        y2[:], w1r_sb[:, OD:2 * OD],
                        psb_sb[:, 2 * b + 1:2 * b + 2], None, op0=OP.mult)
                    nc.vector.tensor_tensor(y1[:], y1[:], y2[:], op=OP.add)
                    nc.vector.scalar_tensor_tensor(
                        y1[:], y1[:], 0.2, y1[:], op0=OP.mult, op1=OP.max)
                    ps_y = psB.tile([D, 128], F32, tag="B")
                    nc.tensor.transpose(ps_y[:], y1[:], id_sb[:])
                    y1T = p3.tile([D, 128], F32, tag="y1T")
                    nc.vector.tensor_copy(y1T[:], ps_y[:])
                    ps_m = psA.tile([128, OD], F32, tag="A")
                    nc.tensor.matmul(ps_m[:], y1T[:], w2s_sb[:],
                                     start=True, stop=True)
                    nc.vector.tensor_tensor(y2[:], ps_m[:], b2b_sb[:], op=OP.add)
                    nc.vector.scalar_tensor_tensor(
                        y2[:], y2[:], 0.01, y2[:], op0=OP.mult, op1=OP.max)
                    nc.vector.tensor_tensor(y2[:], y2[:], y2[:], op=OP.mult)
                    pn = p3.tile([128, 1], F32, tag="pn")
                    nc.vector.tensor_reduce(pn[:], y2[:],
                                            axis=mybir.AxisListType.X, op=OP.add)
                    nc.scalar.activation(pn[:], pn[:], AF.Sqrt)
                    nc.sync.dma_start(pn_out[:, b:b + 1], pn[:])

            # ============ PHASE 4: dense atten ============
            with (
                tc.tile_pool(name="attidx", bufs=1) as attidx,
                tc.tile_pool(name="p4", bufs=2) as p4,
                tc.tile_pool(name="p4b", bufs=2) as p4b,
                tc.tile_pool(name="psC", bufs=2, space="PSUM") as psC,
            ):
                tel_sb = attidx.tile([128, SRCB * BAND // 16], I16)
                tdv_sb = attidx.tile([128, SRCB * BAND // 16], I16)
                tsl_sb = attidx.tile([128, SRCB * BAND // 128], F32)
                tdl_sb = attidx.tile([128, SRCB * BAND // 128], F32)
                nc.sync.dma_start(tel_sb[:], t_el)
                nc.sync.dma_start(tdv_sb[:], t_dv)
                nc.sync.dma_start(tsl_sb[:], t_sl)
                nc.sync.dma_start(tdl_sb[:], t_dl)
                nslots = BAND // 128
                for sb in range(SRCB):
                    gel = p4.tile([128, nslots, ETW], F32, tag="gel")
                    gdv = p4.tile([128, nslots, ETW], F32, tag="gdv")
                    for (o, szz) in cfg["band_subs"]:
                        c0 = (sb * BAND + o) // 16
                        nc.gpsimd.dma_gather(
                            gel[:, o // 128:(o + szz) // 128, :], el_t,
                            tel_sb[:, c0:c0 + szz // 16],
                            num_idxs=szz, num_idxs_reg=szz, elem_size=ETW)
                        nc.gpsimd.dma_gather(
                            gdv[:, o // 128:(o + szz) // 128, :], dst_t,
                            tdv_sb[:, c0:c0 + szz // 16],
                            num_idxs=szz, num_idxs_reg=szz, elem_size=ETW)
                    ee = p4.tile([128, nslots, H], F32, tag="aee")
                    nc.vector.tensor_tensor(ee[:], gel[:, :, 0:H], gdv[:, :, 0:H],
                                            op=OP.add)
                    nc.vector.scalar_tensor_tensor(
                        ee[:], ee[:], 0.2, ee[:], op0=OP.mult, op1=OP.max)
                    nc.vector.tensor_scalar_min(ee[:], ee[:], ECLAMP)
                    nc.scalar.activation(ee[:], ee[:], AF.Exp)
                    nc.vector.tensor_tensor(ee[:], ee[:], gdv[:, :, H:2 * H],
                                            op=OP.mult)
                    val = p4.tile([128, nslots], F32, tag="aval")
                    nc.vector.tensor_reduce(val[:], ee[:],
                                            axis=mybir.AxisListType.X, op=OP.add)
                    band = p4b.tile([128, RPC], F32, tag="band")
                    for ct in range(NCT):
                        ps_a = psC.tile([128, CT], F32, tag="psa")
                        nchunk = B_cap // 128
                        for j in range(nchunk):
                            k = ct * nchunk + j
                            kc = sb * nslots + k
                            ohs = p4.tile([128, 128], MM_DT, tag="ohs")
                            nc.vector.tensor_scalar(
                                ohs[:], io128_sb[:], tsl_sb[:, kc:kc + 1],
                                val[:, k:k + 1], op0=OP.is_equal, op1=OP.mult)
                            ohd = p4.tile([128, CT], MM_DT, tag="ohd")
                            nc.vector.tensor_scalar(
                                ohd[:], ioCT_sb[:], tdl_sb[:, kc:kc + 1],
                                None, op0=OP.is_equal)
                            nc.tensor.matmul(ps_a[:], ohs[:], ohd[:],
                                             start=(j == 0), stop=(j == nchunk - 1))
                        nc.vector.tensor_copy(band[:, ct * CT:(ct + 1) * CT],
                                              ps_a[:])
                    nc.sync.dma_start(
                        atten_loc[sb * 128:(sb + 1) * 128, :], band[:])

    nc.compile()
    return nc



def _assemble(results, cfg):
    N, RPC, OD, BPC = cfg["N"], cfg["RPC"], cfg["OD"], cfg["BPC"]
    n_cores = cfg["n_cores"]
    out = np.concatenate([results[c]["out_rows"] for c in range(n_cores)],
                         axis=0)[:N]
    atten = np.concatenate([results[c]["atten_loc"] for c in range(n_cores)],
                           axis=1)[:N, :N]
    pn = np.concatenate(
        [results[c]["pn_out"].T.reshape(RPC) for c in range(n_cores)])[:N]
    return out.astype(np.float32), atten.astype(np.float32), pn.astype(np.float32)


def kernel(**inputs):
    cfg, in_maps = _preprocess(inputs)
    nc = _build_program(cfg)
    res = bass_utils.run_bass_kernel_spmd(
        nc, in_maps, core_ids=list(range(cfg["n_cores"])))
    return _assemble(res.results, cfg)
